# revision 25
# baseline (speedup 1.0000x reference)
"""Multi-head self-attention Trainium2 kernel (8 NeuronCores).

Sharding: 8 cores = 4 batches x 2 head-groups (8 heads each).
Core c handles batch b=c//2, heads [g*8, (g+1)*8) where g=c%2.
Each core computes a partial output (its heads' contribution to the
output projection); the host sums the two partials per batch and adds bo.

All matmuls run in float32r (fp32 data, ~1 cycle/row vs 4 for fp32,
~1.5e-4 matmul rel err). fp32r matmuls require output base partition 0.

Per-core dataflow (v6 — deep-pipelined, ACT-paced):
  xT [1024, 2048] (= x[b].T), wq/wk/wv [1024, 512], wo [512, 1024]
  A (fused): per 512-token mseg, stream the 8 x k-chunks ONCE (sync DMA
     queue; weights ride the idle Activation DMA queue), then
     A1: QT[p]/KT[p] = w_p.T @ x.T (8 PSUM accs over k-tiles) and
     A2: VS[jt] = [x_jt @ wv | ones] reusing the same chunks.
  B: flattened (qb, pair, key-tile) group loop; per group:
       2 row-packed score MMs (par0 rows 0-63, par1 rows 64-127, K=64,
         concurrent in the PE array) -> stg [128, 1024], 3 PSUM slots,
         emitted TWO groups ahead so exp never stalls on the PE chain
       exp on ScalarE [128, 1024] -> ptg (SBUF f32r)
       2 PV MMs accumulate ct[par] [65, 512] over 16 key tiles (a ones
         col in VS makes row 64 collect sum(exp) = softmax normalizer)
     At each pair end: ct is copied out of PSUM immediately (bank
     release), the softmax normalize runs entirely on SBUF off the
     critical path (recip_approx_fast + GPSIMD partition_broadcast +
     DVE mult -> cth pair tile, two heads stacked for pair-packed C).
  C: out[tokens] = sum_p cth_p.T-slice @ wo_p (K=128, 4-pair PSUM
     accum); C blocks run in the pair-boundary window reusing the just
     released ct PSUM banks (same pool tags), filling the PE bubble
     while the next pair's pipeline restarts; they consume the PREVIOUS
     qb's cth so the normalize latency never gates them.
  PSUM: 3x2 (st) + 2 (ct/po shared) = 8 banks.
"""

import numpy as np

import concourse.bass as bass
import concourse.tile as tile
from concourse import bacc, mybir
from contextlib import ExitStack

P = 128
D = 1024
HD = 512  # head dims per core (8 heads x 64)
NPAIR = 4
NH = 8
F32 = mybir.dt.float32
FR = mybir.dt.float32r


def build_nc(S=2048):
    NKT = D // P          # 8 k-tiles over model dim
    NJT = S // P          # 16 key tiles
    MSEG = 512
    NMSEG = S // MSEG
    QB = 512
    NQB = S // QB

    nc = bacc.Bacc("TRN2", target_bir_lowering=False, debug=False)
    xT = nc.dram_tensor("xT", [D, S], FR, kind="ExternalInput").ap()
    wq = nc.dram_tensor("wq", [D, HD], FR, kind="ExternalInput").ap()
    wk = nc.dram_tensor("wk", [D, HD], FR, kind="ExternalInput").ap()
    wv = nc.dram_tensor("wv", [D, HD], FR, kind="ExternalInput").ap()
    wo = nc.dram_tensor("wo", [HD, D], FR, kind="ExternalInput").ap()
    out = nc.dram_tensor("out", [S, D], F32, kind="ExternalOutput").ap()

    with tile.TileContext(nc) as tc:
        with ExitStack() as persist:
            const_pool = persist.enter_context(tc.tile_pool(name="const", bufs=1))
            data_pool = persist.enter_context(tc.tile_pool(name="data", bufs=1))
            w_pool = persist.enter_context(tc.tile_pool(name="wpool", bufs=1))

            ones8_f32 = const_pool.tile([P, NH], F32, tag="ones8", name="ones8_f32")
            nc.vector.memset(ones8_f32[:], 1.0)

            QT = [data_pool.tile([P, S], FR, tag=f"qt{p}", name=f"qt{p}")
                  for p in range(NPAIR)]
            KT = [data_pool.tile([P, S], FR, tag=f"kt{p}", name=f"kt{p}")
                  for p in range(NPAIR)]
            # [128 tokens, 8 heads x (64 dims + ones col)]
            VS = [data_pool.tile([P, NH * 65], FR, tag=f"vs{j}", name=f"vs{j}")
                  for j in range(NJT)]

            # warm up GPSIMD (first instruction on it costs ~8us)
            gp_warm = const_pool.tile([64, 8], F32, tag="gpw", name="gp_warm")
            nc.gpsimd.partition_broadcast(gp_warm[:], ones8_f32[0:1, 0:8],
                                          channels=64)

            # ---------------- Phase A: projections (fused A1+A2) ----------------
            with ExitStack() as es_a:
                wc_pool = es_a.enter_context(tc.tile_pool(name="wcpool", bufs=1))
                xm_pool = es_a.enter_context(tc.tile_pool(name="xmpool", bufs=2))
                a_ps = es_a.enter_context(
                    tc.tile_pool(name="aps", bufs=8, space="PSUM"))

                # qkv weights stream on the Activation DMA queue (idle during
                # phase A) while x chunks stream on the sync queue, per k-tile
                # so the first matmul only waits for the first chunks of each.
                # wo rides last: phase C needs it only much later.
                wq_c = [wc_pool.tile([P, HD], FR, tag=f"wq{kt}", name=f"wq{kt}")
                        for kt in range(NKT)]
                wk_c = [wc_pool.tile([P, HD], FR, tag=f"wk{kt}", name=f"wk{kt}")
                        for kt in range(NKT)]
                wv_c = [wc_pool.tile([P, HD], FR, tag=f"wv{kt}", name=f"wv{kt}")
                        for kt in range(NKT)]
                for kt in range(NKT):
                    nc.scalar.dma_start(wq_c[kt][:], wq[kt * P:(kt + 1) * P, :])
                    nc.scalar.dma_start(wk_c[kt][:], wk[kt * P:(kt + 1) * P, :])
                for kt in range(NKT):
                    nc.scalar.dma_start(wv_c[kt][:], wv[kt * P:(kt + 1) * P, :])
                wo_p = []
                for p in range(NPAIR):
                    t = w_pool.tile([P, D], FR, tag=f"wo{p}", name=f"wo{p}")
                    nc.scalar.dma_start(t[:], wo[p * P:(p + 1) * P, :])
                    wo_p.append(t)

                for mseg in range(NMSEG):
                    xmc = [xm_pool.tile([P, MSEG], FR, tag=f"xm{kt}", name="xm")
                           for kt in range(NKT)]
                    for kt in range(NKT):
                        nc.sync.dma_start(
                            xmc[kt][:],
                            xT[kt * P:(kt + 1) * P,
                               mseg * MSEG:(mseg + 1) * MSEG])

                    # A1: 8 accumulators ((q|k) x 4 pairs) over 8 k-tiles
                    accs = [a_ps.tile([P, MSEG], F32, tag="acc", name="acc")
                            for _ in range(8)]
                    for kt in range(NKT):
                        for p in range(NPAIR):
                            for ti, wt in ((0, wq_c), (1, wk_c)):
                                nc.tensor.matmul(
                                    accs[p * 2 + ti][:],
                                    lhsT=wt[kt][:, p * P:(p + 1) * P],
                                    rhs=xmc[kt][:],
                                    start=(kt == 0), stop=(kt == NKT - 1))
                    for p in range(NPAIR):
                        nc.vector.tensor_copy(
                            QT[p][:, mseg * MSEG:(mseg + 1) * MSEG],
                            accs[p * 2][:])
                        nc.vector.tensor_copy(
                            KT[p][:, mseg * MSEG:(mseg + 1) * MSEG],
                            accs[p * 2 + 1][:])

                    # A2: V projection for the 4 token tiles of this mseg
                    vaccs = [a_ps.tile([P, HD], F32, tag="acc", name="acc")
                             for _ in range(4)]
                    for kt in range(NKT):
                        for i in range(4):
                            nc.tensor.matmul(
                                vaccs[i][:],
                                lhsT=xmc[kt][:, i * P:(i + 1) * P],
                                rhs=wv_c[kt][:],
                                start=(kt == 0), stop=(kt == NKT - 1))
                    for i in range(4):
                        vsv = VS[mseg * 4 + i].rearrange("p (h c) -> p h c", c=65)
                        nc.vector.tensor_copy(vsv[:, :, 0:64], vaccs[i][:])
                        nc.vector.tensor_copy(vsv[:, :, 64], ones8_f32[:])

            # ------------- Phases B + C: attention + projection -------------
            with ExitStack() as es_b:
                st_ps = es_b.enter_context(
                    tc.tile_pool(name="stps", bufs=1, space="PSUM"))
                ct_ps = es_b.enter_context(
                    tc.tile_pool(name="ctps", bufs=1, space="PSUM"))
                pt_pool = es_b.enter_context(tc.tile_pool(name="ptpool", bufs=1))
                nrm_pool = es_b.enter_context(tc.tile_pool(name="nrmpool", bufs=1))
                cth_pool = es_b.enter_context(tc.tile_pool(name="cthpool", bufs=2))
                po_pool = es_b.enter_context(tc.tile_pool(name="popool", bufs=2))

                def emit_c_rowblock(cth_prev, qb_prev, mtl, st_tag):
                    """One full output row-block (128 tokens x 1024 dims) of
                    phase C, borrowing an idle [128, 1024] slot of the st
                    PSUM pool (two half-width accumulation groups)."""
                    mt = qb_prev * 4 + mtl
                    po = st_ps.tile([P, 1024], F32, tag=st_tag, name="po")
                    for half in range(2):
                        for p in range(NPAIR):
                            nc.tensor.matmul(
                                po[:, half * 512:(half + 1) * 512],
                                lhsT=cth_prev[p][:, mtl * P:(mtl + 1) * P],
                                rhs=wo_p[p][:, half * 512:(half + 1) * 512],
                                start=(p == 0), stop=(p == NPAIR - 1))
                    po_sb = po_pool.tile([P, 1024], F32, tag="posb",
                                         name="po_sb")
                    nc.vector.tensor_copy(po_sb[:], po[:])
                    nc.sync.dma_start(out[mt * P:(mt + 1) * P, :], po_sb[:])

                # Flattened (qb, pair, jt) group loop; score MMs are emitted
                # TWO groups ahead over 3 PSUM slots so exp (the pacing
                # engine) never waits on the PE dependency chain.
                groups = [(qb, p, jt)
                          for qb in range(NQB)
                          for p in range(NPAIR)
                          for jt in range(NJT)]

                def emit_s(g):
                    qb, p, jt = groups[g]
                    stg = st_ps.tile([P, 1024], F32, tag=f"st{g % 3}",
                                     name="stg")
                    for par in range(2):
                        nc.tensor.matmul(
                            stg[:, par * 512:(par + 1) * 512],
                            lhsT=KT[p][par * 64:(par + 1) * 64,
                                       jt * P:(jt + 1) * P],
                            rhs=QT[p][par * 64:(par + 1) * 64,
                                      qb * QB:(qb + 1) * QB],
                            start=True, stop=True)
                    return stg

                cth_prev = None
                qb_prev = -1
                cth_cur = []
                cts = None
                pend = {0: emit_s(0), 1: emit_s(1)}
                for g, (qb, p, jt) in enumerate(groups):
                    stg = pend.pop(g)
                    if jt == 0:
                        cts = [ct_ps.tile([65, QB], F32, tag=t, name=t)
                               for t in ("cte", "cto")]
                    if g + 2 < len(groups) and g + 2 not in pend:
                        pend[g + 2] = emit_s(g + 2)
                    ptg = pt_pool.tile([P, 1024], FR, tag=f"pt{g % 3}",
                                       name="ptg")
                    nc.scalar.activation(
                        ptg[:], stg[:],
                        mybir.ActivationFunctionType.Exp, scale=0.125)
                    for par in range(2):
                        h = 2 * p + par
                        nc.tensor.matmul(
                            cts[par][:],
                            lhsT=VS[jt][:, h * 65:(h + 1) * 65],
                            rhs=ptg[:, par * 512:(par + 1) * 512],
                            start=(jt == 0), stop=(jt == NJT - 1))

                    if jt == NJT - 1:
                        # Copy ct out of PSUM right away (releases the banks
                        # so the next pair's PV can start immediately).
                        ctu = [nrm_pool.tile([65, QB], F32, tag=f"ctu{par}",
                                             name=f"ctu{par}")
                               for par in range(2)]
                        for par in range(2):
                            nc.vector.tensor_copy(ctu[par][:], cts[par][:])
                        # Phase C of the previous qb in the boundary window.
                        # Emit the 3rd-ahead score group FIRST so it isn't
                        # queued behind the C matmuls, then borrow the st
                        # slot whose next score user is a group later.
                        if g + 3 < len(groups) and g + 3 not in pend:
                            pend[g + 3] = emit_s(g + 3)
                        if cth_prev is not None:
                            emit_c_rowblock(cth_prev, qb_prev, p,
                                            f"st{(g + 1) % 3}")
                        # Normalize entirely on SBUF, off the critical path —
                        # cth isn't consumed until the next qb's C blocks.
                        sums = nrm_pool.tile([1, 2 * QB], F32, tag="sums",
                                             name="sums")
                        nc.vector.tensor_copy(sums[:, 0:QB], ctu[0][64:65, :])
                        nc.vector.tensor_copy(sums[:, QB:2 * QB],
                                              ctu[1][64:65, :])
                        rcp = nrm_pool.tile([1, 2 * QB], F32, tag="rcp",
                                            name="rcp")
                        nc.vector.reciprocal_approx_fast(rcp[:], sums[:])
                        cth_t = cth_pool.tile([P, QB], FR, tag=f"cth{p}",
                                              name=f"cth{p}")
                        for par in range(2):
                            bc = nrm_pool.tile([64, QB], F32, tag=f"bc{par}",
                                               name=f"bc{par}")
                            nc.gpsimd.partition_broadcast(
                                bc[:], rcp[:, par * QB:(par + 1) * QB],
                                channels=64)
                            nc.vector.tensor_tensor(
                                cth_t[par * 64:(par + 1) * 64, :],
                                ctu[par][0:64, :], bc[:],
                                mybir.AluOpType.mult)
                        cth_cur.append(cth_t)
                        if p == NPAIR - 1:
                            cth_prev, qb_prev = cth_cur, qb
                            cth_cur = []

                for mtl in range(4):
                    emit_c_rowblock(cth_prev, qb_prev, mtl, f"st{mtl % 3}")
    nc.compile()
    return nc


_NC_CACHE = {}


def _get_nc(S=2048):
    if S not in _NC_CACHE:
        _NC_CACHE[S] = build_nc(S)
    return _NC_CACHE[S]


def kernel(x, Wq, Wk, Wv, Wo, bo):
    from concourse.bass_utils import run_bass_kernel_spmd

    x = np.asarray(x, dtype=np.float32)
    Wq = np.asarray(Wq, dtype=np.float32)
    Wk = np.asarray(Wk, dtype=np.float32)
    Wv = np.asarray(Wv, dtype=np.float32)
    Wo = np.asarray(Wo, dtype=np.float32)
    bo = np.asarray(bo, dtype=np.float32)

    bs, S, d = x.shape
    nc = _get_nc(S)

    in_maps = []
    for c in range(8):
        b, g = divmod(c, 2)
        cols = slice(g * HD, (g + 1) * HD)
        in_maps.append({
            "xT": np.ascontiguousarray(x[b].T),
            "wq": np.ascontiguousarray(Wq[:, cols]),
            "wk": np.ascontiguousarray(Wk[:, cols]),
            "wv": np.ascontiguousarray(Wv[:, cols]),
            "wo": np.ascontiguousarray(Wo[cols, :]),
        })

    res = run_bass_kernel_spmd(nc, in_maps, core_ids=list(range(8)))
    outp = np.empty((bs, S, d), dtype=np.float32)
    for b in range(bs):
        outp[b] = res.results[2 * b]["out"] + res.results[2 * b + 1]["out"] + bo
    return outp


# revision 27
# speedup vs baseline: 1.0219x; 1.0219x over previous
"""Multi-head self-attention Trainium2 kernel (8 NeuronCores).

Sharding: 8 cores = 4 batches x 2 head-groups (8 heads each).
Core c handles batch b=c//2, heads [g*8, (g+1)*8) where g=c%2.
Each core computes a partial output (its heads' contribution to the
output projection); the host sums the two partials per batch and adds bo.

All matmuls run in float32r (fp32 data, ~1 cycle/row vs 4 for fp32,
~1.5e-4 matmul rel err). fp32r matmuls require output base partition 0.

Per-core dataflow (v6 — deep-pipelined, ACT-paced):
  xT [1024, 2048] (= x[b].T), wq/wk/wv [1024, 512], wo [512, 1024]
  A (fused): per 512-token mseg, stream the 8 x k-chunks ONCE (sync DMA
     queue; weights ride the idle Activation DMA queue), then
     A1: QT[p]/KT[p] = w_p.T @ x.T (8 PSUM accs over k-tiles) and
     A2: VS[jt] = [x_jt @ wv | ones] reusing the same chunks.
  B: flattened (qb, pair, key-tile) group loop; per group:
       2 row-packed score MMs (par0 rows 0-63, par1 rows 64-127, K=64,
         concurrent in the PE array) -> stg [128, 1024], 3 PSUM slots,
         emitted TWO groups ahead so exp never stalls on the PE chain
       exp on ScalarE [128, 1024] -> ptg (SBUF f32r)
       2 PV MMs accumulate ct[par] [65, 512] over 16 key tiles (a ones
         col in VS makes row 64 collect sum(exp) = softmax normalizer)
     At each pair end: ct is copied out of PSUM immediately (bank
     release), the softmax normalize runs entirely on SBUF off the
     critical path (recip_approx_fast + GPSIMD partition_broadcast +
     DVE mult -> cth pair tile, two heads stacked for pair-packed C).
  C: out[tokens] = sum_p cth_p.T-slice @ wo_p (K=128, 4-pair PSUM
     accum); C blocks run in the pair-boundary window reusing the just
     released ct PSUM banks (same pool tags), filling the PE bubble
     while the next pair's pipeline restarts; they consume the PREVIOUS
     qb's cth so the normalize latency never gates them.
  PSUM: 3x2 (st) + 2 (ct/po shared) = 8 banks.
"""

import numpy as np

import concourse.bass as bass
import concourse.tile as tile
from concourse import bacc, mybir
from contextlib import ExitStack

P = 128
D = 1024
HD = 512  # head dims per core (8 heads x 64)
NPAIR = 4
NH = 8
F32 = mybir.dt.float32
FR = mybir.dt.float32r


def build_nc(S=2048):
    NKT = D // P          # 8 k-tiles over model dim
    NJT = S // P          # 16 key tiles
    MSEG = 512
    NMSEG = S // MSEG
    QB = 512
    NQB = S // QB

    nc = bacc.Bacc("TRN2", target_bir_lowering=False, debug=False)
    xT = nc.dram_tensor("xT", [D, S], FR, kind="ExternalInput").ap()
    wq = nc.dram_tensor("wq", [D, HD], FR, kind="ExternalInput").ap()
    wk = nc.dram_tensor("wk", [D, HD], FR, kind="ExternalInput").ap()
    wv = nc.dram_tensor("wv", [D, HD], FR, kind="ExternalInput").ap()
    wo = nc.dram_tensor("wo", [HD, D], FR, kind="ExternalInput").ap()
    out = nc.dram_tensor("out", [S, D], F32, kind="ExternalOutput").ap()

    with tile.TileContext(nc) as tc:
        with ExitStack() as persist:
            const_pool = persist.enter_context(tc.tile_pool(name="const", bufs=1))
            data_pool = persist.enter_context(tc.tile_pool(name="data", bufs=1))
            w_pool = persist.enter_context(tc.tile_pool(name="wpool", bufs=1))

            ones8_f32 = const_pool.tile([P, NH], F32, tag="ones8", name="ones8_f32")
            nc.vector.memset(ones8_f32[:], 1.0)

            QT = [data_pool.tile([P, S], FR, tag=f"qt{p}", name=f"qt{p}")
                  for p in range(NPAIR)]
            KT = [data_pool.tile([P, S], FR, tag=f"kt{p}", name=f"kt{p}")
                  for p in range(NPAIR)]
            # [128 tokens, 8 heads x (64 dims + ones col)]
            VS = [data_pool.tile([P, NH * 65], FR, tag=f"vs{j}", name=f"vs{j}")
                  for j in range(NJT)]

            # warm up GPSIMD (first instruction on it costs ~8us)
            gp_warm = const_pool.tile([64, 8], F32, tag="gpw", name="gp_warm")
            nc.gpsimd.partition_broadcast(gp_warm[:], ones8_f32[0:1, 0:8],
                                          channels=64)

            # ---------------- Phase A: projections (fused A1+A2) ----------------
            with ExitStack() as es_a:
                wc_pool = es_a.enter_context(tc.tile_pool(name="wcpool", bufs=1))
                xm_pool = es_a.enter_context(tc.tile_pool(name="xmpool", bufs=2))
                a_ps = es_a.enter_context(
                    tc.tile_pool(name="aps", bufs=8, space="PSUM"))

                # qkv weights stream on the Activation DMA queue (idle during
                # phase A) while x chunks stream on the sync queue, per k-tile
                # so the first matmul only waits for the first chunks of each.
                # wo rides last: phase C needs it only much later.
                wq_c = [wc_pool.tile([P, HD], FR, tag=f"wq{kt}", name=f"wq{kt}")
                        for kt in range(NKT)]
                wk_c = [wc_pool.tile([P, HD], FR, tag=f"wk{kt}", name=f"wk{kt}")
                        for kt in range(NKT)]
                wv_c = [wc_pool.tile([P, HD], FR, tag=f"wv{kt}", name=f"wv{kt}")
                        for kt in range(NKT)]
                for kt in range(NKT):
                    nc.scalar.dma_start(wq_c[kt][:], wq[kt * P:(kt + 1) * P, :])
                    nc.scalar.dma_start(wk_c[kt][:], wk[kt * P:(kt + 1) * P, :])
                for kt in range(NKT):
                    nc.scalar.dma_start(wv_c[kt][:], wv[kt * P:(kt + 1) * P, :])
                wo_p = []
                for p in range(NPAIR):
                    t = w_pool.tile([P, D], FR, tag=f"wo{p}", name=f"wo{p}")
                    nc.scalar.dma_start(t[:], wo[p * P:(p + 1) * P, :])
                    wo_p.append(t)

                for mseg in range(NMSEG):
                    xmc = [xm_pool.tile([P, MSEG], FR, tag=f"xm{kt}", name="xm")
                           for kt in range(NKT)]
                    for kt in range(NKT):
                        nc.sync.dma_start(
                            xmc[kt][:],
                            xT[kt * P:(kt + 1) * P,
                               mseg * MSEG:(mseg + 1) * MSEG])

                    # A1: 8 accumulators ((q|k) x 4 pairs) over 8 k-tiles
                    accs = [a_ps.tile([P, MSEG], F32, tag="acc", name="acc")
                            for _ in range(8)]
                    for kt in range(NKT):
                        for p in range(NPAIR):
                            for ti, wt in ((0, wq_c), (1, wk_c)):
                                nc.tensor.matmul(
                                    accs[p * 2 + ti][:],
                                    lhsT=wt[kt][:, p * P:(p + 1) * P],
                                    rhs=xmc[kt][:],
                                    start=(kt == 0), stop=(kt == NKT - 1))
                    for p in range(NPAIR):
                        nc.vector.tensor_copy(
                            QT[p][:, mseg * MSEG:(mseg + 1) * MSEG],
                            accs[p * 2][:])
                        nc.vector.tensor_copy(
                            KT[p][:, mseg * MSEG:(mseg + 1) * MSEG],
                            accs[p * 2 + 1][:])

                    # A2: V projection for the 4 token tiles of this mseg
                    vaccs = [a_ps.tile([P, HD], F32, tag="acc", name="acc")
                             for _ in range(4)]
                    for kt in range(NKT):
                        for i in range(4):
                            nc.tensor.matmul(
                                vaccs[i][:],
                                lhsT=xmc[kt][:, i * P:(i + 1) * P],
                                rhs=wv_c[kt][:],
                                start=(kt == 0), stop=(kt == NKT - 1))
                    for i in range(4):
                        vsv = VS[mseg * 4 + i].rearrange("p (h c) -> p h c", c=65)
                        nc.vector.tensor_copy(vsv[:, :, 0:64], vaccs[i][:])
                        nc.vector.tensor_copy(vsv[:, :, 64], ones8_f32[:])

            # ------------- Phases B + C: attention + projection -------------
            with ExitStack() as es_b:
                st_ps = es_b.enter_context(
                    tc.tile_pool(name="stps", bufs=1, space="PSUM"))
                ct_ps = es_b.enter_context(
                    tc.tile_pool(name="ctps", bufs=1, space="PSUM"))
                pt_pool = es_b.enter_context(tc.tile_pool(name="ptpool", bufs=1))
                nrm_pool = es_b.enter_context(tc.tile_pool(name="nrmpool", bufs=1))
                cth_pool = es_b.enter_context(tc.tile_pool(name="cthpool", bufs=2))
                po_pool = es_b.enter_context(tc.tile_pool(name="popool", bufs=2))

                def emit_c_rowblock(cth_prev, qb_prev, mtl, st_tag):
                    """One full output row-block (128 tokens x 1024 dims) of
                    phase C, borrowing an idle [128, 1024] slot of the st
                    PSUM pool (two half-width accumulation groups)."""
                    mt = qb_prev * 4 + mtl
                    po = st_ps.tile([P, 1024], F32, tag=st_tag, name="po")
                    for half in range(2):
                        for p in range(NPAIR):
                            nc.tensor.matmul(
                                po[:, half * 512:(half + 1) * 512],
                                lhsT=cth_prev[p][:, mtl * P:(mtl + 1) * P],
                                rhs=wo_p[p][:, half * 512:(half + 1) * 512],
                                start=(p == 0), stop=(p == NPAIR - 1))
                    po_sb = po_pool.tile([P, 1024], F32, tag="posb",
                                         name="po_sb")
                    nc.vector.tensor_copy(po_sb[:], po[:])
                    nc.sync.dma_start(out[mt * P:(mt + 1) * P, :], po_sb[:])

                # Flattened (qb, pair, jt) group loop; score MMs are emitted
                # TWO groups ahead over 3 PSUM slots so exp (the pacing
                # engine) never waits on the PE dependency chain.
                groups = [(qb, p, jt)
                          for qb in range(NQB)
                          for p in range(NPAIR)
                          for jt in range(NJT)]

                def emit_s(g):
                    qb, p, jt = groups[g]
                    stg = st_ps.tile([P, 1024], F32, tag=f"st{g % 3}",
                                     name="stg")
                    for par in range(2):
                        nc.tensor.matmul(
                            stg[:, par * 512:(par + 1) * 512],
                            lhsT=KT[p][par * 64:(par + 1) * 64,
                                       jt * P:(jt + 1) * P],
                            rhs=QT[p][par * 64:(par + 1) * 64,
                                      qb * QB:(qb + 1) * QB],
                            start=True, stop=True)
                    return stg

                cth_prev = None
                qb_prev = -1
                cth_cur = []
                cts = None
                pend = {0: emit_s(0), 1: emit_s(1), 2: emit_s(2)}
                for g, (qb, p, jt) in enumerate(groups):
                    stg = pend.pop(g)
                    if jt == 0:
                        cts = [ct_ps.tile([65, QB], F32, tag=t, name=t)
                               for t in ("cte", "cto")]
                    if g + 3 < len(groups) and g + 3 not in pend:
                        pend[g + 3] = emit_s(g + 3)
                    ptg = pt_pool.tile([P, 1024], FR, tag=f"pt{g % 3}",
                                       name="ptg")
                    nc.scalar.activation(
                        ptg[:], stg[:],
                        mybir.ActivationFunctionType.Exp, scale=0.125)
                    for par in range(2):
                        h = 2 * p + par
                        nc.tensor.matmul(
                            cts[par][:],
                            lhsT=VS[jt][:, h * 65:(h + 1) * 65],
                            rhs=ptg[:, par * 512:(par + 1) * 512],
                            start=(jt == 0), stop=(jt == NJT - 1))

                    if jt == NJT - 1:
                        # Copy ct out of PSUM right away (releases the banks
                        # so the next pair's PV can start immediately).
                        ctu = [nrm_pool.tile([65, QB], F32, tag=f"ctu{par}",
                                             name=f"ctu{par}")
                               for par in range(2)]
                        for par in range(2):
                            nc.vector.tensor_copy(ctu[par][:], cts[par][:])
                        # Phase C of the previous qb in the boundary window;
                        # the current group's st slot (tag g%3) frees as soon
                        # as this group's exp is done, well before the next
                        # score MM wants it back.
                        if cth_prev is not None:
                            emit_c_rowblock(cth_prev, qb_prev, p, f"st{g % 3}")
                        # Normalize entirely on SBUF, off the critical path —
                        # cth isn't consumed until the next qb's C blocks.
                        sums = nrm_pool.tile([1, 2 * QB], F32, tag="sums",
                                             name="sums")
                        nc.vector.tensor_copy(sums[:, 0:QB], ctu[0][64:65, :])
                        nc.vector.tensor_copy(sums[:, QB:2 * QB],
                                              ctu[1][64:65, :])
                        rcp = nrm_pool.tile([1, 2 * QB], F32, tag="rcp",
                                            name="rcp")
                        nc.vector.reciprocal_approx_fast(rcp[:], sums[:])
                        cth_t = cth_pool.tile([P, QB], FR, tag=f"cth{p}",
                                              name=f"cth{p}")
                        for par in range(2):
                            bc = nrm_pool.tile([64, QB], F32, tag=f"bc{par}",
                                               name=f"bc{par}")
                            nc.gpsimd.partition_broadcast(
                                bc[:], rcp[:, par * QB:(par + 1) * QB],
                                channels=64)
                            nc.vector.tensor_tensor(
                                cth_t[par * 64:(par + 1) * 64, :],
                                ctu[par][0:64, :], bc[:],
                                mybir.AluOpType.mult)
                        cth_cur.append(cth_t)
                        if p == NPAIR - 1:
                            cth_prev, qb_prev = cth_cur, qb
                            cth_cur = []

                for mtl in range(4):
                    emit_c_rowblock(cth_prev, qb_prev, mtl, f"st{mtl % 3}")
    nc.compile()
    return nc


_NC_CACHE = {}


def _get_nc(S=2048):
    if S not in _NC_CACHE:
        _NC_CACHE[S] = build_nc(S)
    return _NC_CACHE[S]


def kernel(x, Wq, Wk, Wv, Wo, bo):
    from concourse.bass_utils import run_bass_kernel_spmd

    x = np.asarray(x, dtype=np.float32)
    Wq = np.asarray(Wq, dtype=np.float32)
    Wk = np.asarray(Wk, dtype=np.float32)
    Wv = np.asarray(Wv, dtype=np.float32)
    Wo = np.asarray(Wo, dtype=np.float32)
    bo = np.asarray(bo, dtype=np.float32)

    bs, S, d = x.shape
    nc = _get_nc(S)

    in_maps = []
    for c in range(8):
        b, g = divmod(c, 2)
        cols = slice(g * HD, (g + 1) * HD)
        in_maps.append({
            "xT": np.ascontiguousarray(x[b].T),
            "wq": np.ascontiguousarray(Wq[:, cols]),
            "wk": np.ascontiguousarray(Wk[:, cols]),
            "wv": np.ascontiguousarray(Wv[:, cols]),
            "wo": np.ascontiguousarray(Wo[cols, :]),
        })

    res = run_bass_kernel_spmd(nc, in_maps, core_ids=list(range(8)))
    outp = np.empty((bs, S, d), dtype=np.float32)
    for b in range(bs):
        outp[b] = res.results[2 * b]["out"] + res.results[2 * b + 1]["out"] + bo
    return outp


# revision 28
# speedup vs baseline: 1.0377x; 1.0154x over previous
"""Multi-head self-attention Trainium2 kernel (8 NeuronCores).

Sharding: 8 cores = 4 batches x 2 head-groups (8 heads each).
Core c handles batch b=c//2, heads [g*8, (g+1)*8) where g=c%2.
Each core computes a partial output (its heads' contribution to the
output projection); the host sums the two partials per batch and adds bo.

All matmuls run in float32r (fp32 data, ~1 cycle/row vs 4 for fp32,
~1.5e-4 matmul rel err). fp32r matmuls require output base partition 0.

Per-core dataflow (v6 — deep-pipelined, ACT-paced):
  xT [1024, 2048] (= x[b].T), wq/wk/wv [1024, 512], wo [512, 1024]
  A (fused): per 512-token mseg, stream the 8 x k-chunks ONCE (sync DMA
     queue; weights ride the idle Activation DMA queue), then
     A1: QT[p]/KT[p] = w_p.T @ x.T (8 PSUM accs over k-tiles) and
     A2: VS[jt] = [x_jt @ wv | ones] reusing the same chunks.
  B: flattened (qb, pair, key-tile) group loop; per group:
       2 row-packed score MMs (par0 rows 0-63, par1 rows 64-127, K=64,
         concurrent in the PE array) -> stg [128, 1024], 3 PSUM slots,
         emitted TWO groups ahead so exp never stalls on the PE chain
       exp on ScalarE [128, 1024] -> ptg (SBUF f32r)
       2 PV MMs accumulate ct[par] [65, 512] over 16 key tiles (a ones
         col in VS makes row 64 collect sum(exp) = softmax normalizer)
     At each pair end: ct is copied out of PSUM immediately (bank
     release), the softmax normalize runs entirely on SBUF off the
     critical path (recip_approx_fast + GPSIMD partition_broadcast +
     DVE mult -> cth pair tile, two heads stacked for pair-packed C).
  C: out[tokens] = sum_p cth_p.T-slice @ wo_p (K=128, 4-pair PSUM
     accum); C blocks run in the pair-boundary window reusing the just
     released ct PSUM banks (same pool tags), filling the PE bubble
     while the next pair's pipeline restarts; they consume the PREVIOUS
     qb's cth so the normalize latency never gates them.
  PSUM: 3x2 (st) + 2 (ct/po shared) = 8 banks.
"""

import numpy as np

import concourse.bass as bass
import concourse.tile as tile
from concourse import bacc, mybir
from contextlib import ExitStack

P = 128
D = 1024
HD = 512  # head dims per core (8 heads x 64)
NPAIR = 4
NH = 8
F32 = mybir.dt.float32
FR = mybir.dt.float32r


def build_nc(S=2048):
    NKT = D // P          # 8 k-tiles over model dim
    NJT = S // P          # 16 key tiles
    MSEG = 512
    NMSEG = S // MSEG
    QB = 512
    NQB = S // QB

    nc = bacc.Bacc("TRN2", target_bir_lowering=False, debug=False)
    xT = nc.dram_tensor("xT", [D, S], FR, kind="ExternalInput").ap()
    wq = nc.dram_tensor("wq", [D, HD], FR, kind="ExternalInput").ap()
    wk = nc.dram_tensor("wk", [D, HD], FR, kind="ExternalInput").ap()
    wv = nc.dram_tensor("wv", [D, HD], FR, kind="ExternalInput").ap()
    wo = nc.dram_tensor("wo", [HD, D], FR, kind="ExternalInput").ap()
    out = nc.dram_tensor("out", [S, D], F32, kind="ExternalOutput").ap()

    with tile.TileContext(nc) as tc:
        with ExitStack() as persist:
            const_pool = persist.enter_context(tc.tile_pool(name="const", bufs=1))
            data_pool = persist.enter_context(tc.tile_pool(name="data", bufs=1))
            w_pool = persist.enter_context(tc.tile_pool(name="wpool", bufs=1))

            ones8_f32 = const_pool.tile([P, NH], F32, tag="ones8", name="ones8_f32")
            nc.vector.memset(ones8_f32[:], 1.0)

            QT = [data_pool.tile([P, S], FR, tag=f"qt{p}", name=f"qt{p}")
                  for p in range(NPAIR)]
            KT = [data_pool.tile([P, S], FR, tag=f"kt{p}", name=f"kt{p}")
                  for p in range(NPAIR)]
            # [128 tokens, 8 heads x (64 dims + ones col)]
            VS = [data_pool.tile([P, NH * 65], FR, tag=f"vs{j}", name=f"vs{j}")
                  for j in range(NJT)]

            # warm up GPSIMD (first instruction on it costs ~8us)
            gp_warm = const_pool.tile([64, 8], F32, tag="gpw", name="gp_warm")
            nc.gpsimd.partition_broadcast(gp_warm[:], ones8_f32[0:1, 0:8],
                                          channels=64)

            # ---------------- Phase A: projections (fused A1+A2) ----------------
            with ExitStack() as es_a:
                wc_pool = es_a.enter_context(tc.tile_pool(name="wcpool", bufs=1))
                xm_pool = es_a.enter_context(tc.tile_pool(name="xmpool", bufs=2))
                a_ps = es_a.enter_context(
                    tc.tile_pool(name="aps", bufs=8, space="PSUM"))

                # qkv weights stream on the Activation DMA queue (idle during
                # phase A) while x chunks stream on the sync queue, per k-tile
                # so the first matmul only waits for the first chunks of each.
                # wo rides last: phase C needs it only much later.
                wq_c = [wc_pool.tile([P, HD], FR, tag=f"wq{kt}", name=f"wq{kt}")
                        for kt in range(NKT)]
                wk_c = [wc_pool.tile([P, HD], FR, tag=f"wk{kt}", name=f"wk{kt}")
                        for kt in range(NKT)]
                wv_c = [wc_pool.tile([P, HD], FR, tag=f"wv{kt}", name=f"wv{kt}")
                        for kt in range(NKT)]
                for kt in range(NKT):
                    nc.scalar.dma_start(wq_c[kt][:], wq[kt * P:(kt + 1) * P, :])
                    nc.scalar.dma_start(wk_c[kt][:], wk[kt * P:(kt + 1) * P, :])
                for kt in range(NKT):
                    nc.scalar.dma_start(wv_c[kt][:], wv[kt * P:(kt + 1) * P, :])
                wo_p = []
                for p in range(NPAIR):
                    t = w_pool.tile([P, D], FR, tag=f"wo{p}", name=f"wo{p}")
                    nc.scalar.dma_start(t[:], wo[p * P:(p + 1) * P, :])
                    wo_p.append(t)

                for mseg in range(NMSEG):
                    xmc = [xm_pool.tile([P, MSEG], FR, tag=f"xm{kt}", name="xm")
                           for kt in range(NKT)]
                    for kt in range(NKT):
                        nc.sync.dma_start(
                            xmc[kt][:],
                            xT[kt * P:(kt + 1) * P,
                               mseg * MSEG:(mseg + 1) * MSEG])

                    # A1: 8 accumulators ((q|k) x 4 pairs) over 8 k-tiles
                    accs = [a_ps.tile([P, MSEG], F32, tag="acc", name="acc")
                            for _ in range(8)]
                    for kt in range(NKT):
                        for p in range(NPAIR):
                            for ti, wt in ((0, wq_c), (1, wk_c)):
                                nc.tensor.matmul(
                                    accs[p * 2 + ti][:],
                                    lhsT=wt[kt][:, p * P:(p + 1) * P],
                                    rhs=xmc[kt][:],
                                    start=(kt == 0), stop=(kt == NKT - 1))
                    for p in range(NPAIR):
                        nc.vector.tensor_copy(
                            QT[p][:, mseg * MSEG:(mseg + 1) * MSEG],
                            accs[p * 2][:])
                        nc.vector.tensor_copy(
                            KT[p][:, mseg * MSEG:(mseg + 1) * MSEG],
                            accs[p * 2 + 1][:])

                    # A2: V projection for the 4 token tiles of this mseg
                    vaccs = [a_ps.tile([P, HD], F32, tag="acc", name="acc")
                             for _ in range(4)]
                    for kt in range(NKT):
                        for i in range(4):
                            nc.tensor.matmul(
                                vaccs[i][:],
                                lhsT=xmc[kt][:, i * P:(i + 1) * P],
                                rhs=wv_c[kt][:],
                                start=(kt == 0), stop=(kt == NKT - 1))
                    for i in range(4):
                        vsv = VS[mseg * 4 + i].rearrange("p (h c) -> p h c", c=65)
                        nc.vector.tensor_copy(vsv[:, :, 0:64], vaccs[i][:])
                        nc.vector.tensor_copy(vsv[:, :, 64], ones8_f32[:])

            # ------------- Phases B + C: attention + projection -------------
            with ExitStack() as es_b:
                st_ps = es_b.enter_context(
                    tc.tile_pool(name="stps", bufs=1, space="PSUM"))
                ct_ps = es_b.enter_context(
                    tc.tile_pool(name="ctps", bufs=1, space="PSUM"))
                pt_pool = es_b.enter_context(tc.tile_pool(name="ptpool", bufs=1))
                nrm_pool = es_b.enter_context(tc.tile_pool(name="nrmpool", bufs=1))
                cth_pool = es_b.enter_context(tc.tile_pool(name="cthpool", bufs=2))
                po_pool = es_b.enter_context(tc.tile_pool(name="popool", bufs=2))

                def emit_c_rowblock(cth_prev, qb_prev, mtl, st_tag):
                    """One full output row-block (128 tokens x 1024 dims) of
                    phase C, borrowing an idle [128, 1024] slot of the st
                    PSUM pool (two half-width accumulation groups)."""
                    mt = qb_prev * 4 + mtl
                    po = st_ps.tile([P, 1024], F32, tag=st_tag, name="po")
                    for half in range(2):
                        for p in range(NPAIR):
                            nc.tensor.matmul(
                                po[:, half * 512:(half + 1) * 512],
                                lhsT=cth_prev[p][:, mtl * P:(mtl + 1) * P],
                                rhs=wo_p[p][:, half * 512:(half + 1) * 512],
                                start=(p == 0), stop=(p == NPAIR - 1))
                    po_sb = po_pool.tile([P, 1024], F32, tag="posb",
                                         name="po_sb")
                    nc.vector.tensor_copy(po_sb[:], po[:])
                    nc.sync.dma_start(out[mt * P:(mt + 1) * P, :], po_sb[:])

                # Flattened (qb, pair, jt) group loop; score MMs are emitted
                # TWO groups ahead over 3 PSUM slots so exp (the pacing
                # engine) never waits on the PE dependency chain.
                groups = [(qb, p, jt)
                          for qb in range(NQB)
                          for p in range(NPAIR)
                          for jt in range(NJT)]

                def emit_s(g):
                    qb, p, jt = groups[g]
                    stg = st_ps.tile([P, 1024], F32, tag=f"st{g % 3}",
                                     name="stg")
                    for par in range(2):
                        nc.tensor.matmul(
                            stg[:, par * 512:(par + 1) * 512],
                            lhsT=KT[p][par * 64:(par + 1) * 64,
                                       jt * P:(jt + 1) * P],
                            rhs=QT[p][par * 64:(par + 1) * 64,
                                      qb * QB:(qb + 1) * QB],
                            start=True, stop=True)
                    return stg

                cth_prev = None
                qb_prev = -1
                cth_cur = []
                cts = None
                pend = {0: emit_s(0), 1: emit_s(1)}
                for g, (qb, p, jt) in enumerate(groups):
                    stg = pend.pop(g)
                    if jt == 0:
                        cts = [ct_ps.tile([65, QB], F32, tag=t, name=t)
                               for t in ("cte", "cto")]
                    if g + 2 < len(groups) and g + 2 not in pend:
                        pend[g + 2] = emit_s(g + 2)
                    ptg = pt_pool.tile([P, 1024], FR, tag=f"pt{g % 3}",
                                       name="ptg")
                    nc.scalar.activation(
                        ptg[:], stg[:],
                        mybir.ActivationFunctionType.Exp, scale=0.125)
                    for par in range(2):
                        h = 2 * p + par
                        nc.tensor.matmul(
                            cts[par][:],
                            lhsT=VS[jt][:, h * 65:(h + 1) * 65],
                            rhs=ptg[:, par * 512:(par + 1) * 512],
                            start=(jt == 0), stop=(jt == NJT - 1))

                    if jt == NJT - 1:
                        # Copy ct out of PSUM right away (releases the banks
                        # so the next pair's PV can start immediately).
                        ctu = [nrm_pool.tile([65, QB], F32, tag=f"ctu{par}",
                                             name=f"ctu{par}")
                               for par in range(2)]
                        for par in range(2):
                            nc.vector.tensor_copy(ctu[par][:], cts[par][:])
                        # Phase C of the previous qb in the boundary window;
                        # the current group's st slot (tag g%3) frees as soon
                        # as this group's exp is done, well before the next
                        # score MM wants it back.
                        if cth_prev is not None:
                            emit_c_rowblock(cth_prev, qb_prev, p, f"st{g % 3}")
                        # Normalize entirely on SBUF, off the critical path —
                        # cth isn't consumed until the next qb's C blocks.
                        sums = nrm_pool.tile([1, 2 * QB], F32, tag="sums",
                                             name="sums")
                        nc.vector.tensor_copy(sums[:, 0:QB], ctu[0][64:65, :])
                        nc.vector.tensor_copy(sums[:, QB:2 * QB],
                                              ctu[1][64:65, :])
                        rcp = nrm_pool.tile([1, 2 * QB], F32, tag="rcp",
                                            name="rcp")
                        nc.vector.reciprocal_approx_fast(rcp[:], sums[:])
                        cth_t = cth_pool.tile([P, QB], FR, tag=f"cth{p}",
                                              name=f"cth{p}")
                        for par in range(2):
                            bc = nrm_pool.tile([64, QB], F32, tag=f"bc{par}",
                                               name=f"bc{par}")
                            nc.gpsimd.partition_broadcast(
                                bc[:], rcp[:, par * QB:(par + 1) * QB],
                                channels=64)
                            nc.vector.tensor_tensor(
                                cth_t[par * 64:(par + 1) * 64, :],
                                ctu[par][0:64, :], bc[:],
                                mybir.AluOpType.mult)
                        cth_cur.append(cth_t)
                        if p == NPAIR - 1:
                            cth_prev, qb_prev = cth_cur, qb
                            cth_cur = []

                for mtl in range(4):
                    emit_c_rowblock(cth_prev, qb_prev, mtl, f"st{mtl % 3}")
    nc.compile()
    return nc


_NC_CACHE = {}


def _get_nc(S=2048):
    if S not in _NC_CACHE:
        _NC_CACHE[S] = build_nc(S)
    return _NC_CACHE[S]


def kernel(x, Wq, Wk, Wv, Wo, bo):
    from concourse.bass_utils import run_bass_kernel_spmd

    x = np.asarray(x, dtype=np.float32)
    Wq = np.asarray(Wq, dtype=np.float32)
    Wk = np.asarray(Wk, dtype=np.float32)
    Wv = np.asarray(Wv, dtype=np.float32)
    Wo = np.asarray(Wo, dtype=np.float32)
    bo = np.asarray(bo, dtype=np.float32)

    bs, S, d = x.shape
    nc = _get_nc(S)

    in_maps = []
    for c in range(8):
        b, g = divmod(c, 2)
        cols = slice(g * HD, (g + 1) * HD)
        in_maps.append({
            "xT": np.ascontiguousarray(x[b].T),
            "wq": np.ascontiguousarray(Wq[:, cols]),
            "wk": np.ascontiguousarray(Wk[:, cols]),
            "wv": np.ascontiguousarray(Wv[:, cols]),
            "wo": np.ascontiguousarray(Wo[cols, :]),
        })

    res = run_bass_kernel_spmd(nc, in_maps, core_ids=list(range(8)))
    outp = np.empty((bs, S, d), dtype=np.float32)
    for b in range(bs):
        outp[b] = res.results[2 * b]["out"] + res.results[2 * b + 1]["out"] + bo
    return outp
